# revision 16
# baseline (speedup 1.0000x reference)
"""YOLO-style loss kernel for Trainium2, 8-core data-parallel.

Strategy: shard the 16384 batch across 8 cores (2048 each = 100352 grid
cells). Each core streams its [cells, 30] pred/target arrays through SBUF
(fp32 in HBM, cast to bf16 during the DMA), computes every loss term as a
mask-in-{0,1} times value, concatenates the masked values into one scratch
strip per tile, and reduces with the scalar engine's
ACTIVATE(Square, scale=sqrt(term_weight), accum_out=...) — three accum ops
per tile, one per loss weight (1, 0.5, 5), so term weights stay exact fp32
while masks are exact binary. The host sums the 8x[128, NT*3] partials and
divides by N.

Per-cell math (channels [x0,y0,w0,h0,c0, x1,y1,w1,h1,c1, 20 class]):
  obj  = t4 > 0, noo = t4 == 0, d = p - t
  noobj term   = 0.5*noo*(d4^2 + d9^2)
  class term   = obj * sum_cls d^2
  iou(i,j) of pred box i vs target box j from xyxy at x/7 +- w/2
  g_j = iou(1,j) > iou(0,j) (argmax), m_j = max_i iou(i,j)
  conf targets c0 = m1 + g1*(m0-m1), c1 = m0 - g1*(m0-m1) (last-write-wins)
  resp_0 = obj*(1-g0*g1), resp_1 = obj*(g0+g1-g0*g1)
  contain term = resp_b*(pconf_b - c_b)^2
  loc term     = 5*resp_b*(dxy^2 + (sqrt(pwh+eps)-sqrt(twh+eps))^2)
"""

import math

import numpy as np
import concourse.bass as bass
import concourse.tile as tile
from concourse import mybir
from concourse.bass_utils import run_bass_kernel_spmd

F32 = mybir.dt.float32
BF16 = mybir.dt.bfloat16
Alu = mybir.AluOpType
Act = mybir.ActivationFunctionType

# problem constants (hardcoded per harness contract)
BATCH = 16384
S = 7
D = 30
N_CORES = 8
B_PER = BATCH // N_CORES            # 2048
K_CORE = B_PER * S * S              # 100352 cells/core
P = 128
CELLS_PER_PART = K_CORE // P        # 784
NT = 4                              # tiles per core
CPP = CELLS_PER_PART // NT          # cells per partition per tile
INV_S = 1.0 / 7.0
EPS = 1e-6
SQRT5 = math.sqrt(5.0)
SQRT_HALF = math.sqrt(0.5)
NGROUPS = 3                         # accum scale groups: 1.0 / sqrt(.5) / sqrt(5)


def split_sync_waits(nc, max_attached=1):
    """This container's walrus build rejects >1 semaphore wait attached to an
    instruction. Hoist the extras into standalone EventSemaphore wait
    instructions (what raw-bass wait_ge emits), which it accepts."""
    n = 0
    for func in nc.m.functions:
        for bb in func.blocks:
            insts = list(bb.instructions)
            out = []
            changed = False
            for inst in insts:
                si = inst.sync_info
                if si is not None and len(si.on_wait) > max_attached:
                    waits = list(si.on_wait)
                    keep, hoist = waits[:max_attached], waits[max_attached:]
                    for k, w in enumerate(hoist):
                        wi = mybir.InstEventSemaphore(
                            name=f"{inst.name}-hw{k}", ins=[], outs=[]
                        )
                        wi.engine = inst.engine
                        wi.sync_info = mybir.SyncInfo(on_wait=[w], on_update=[])
                        nc.register_instruction(wi, overwrite=True)
                        out.append(wi)
                        n += 1
                    inst.sync_info = mybir.SyncInfo(
                        on_wait=keep, on_update=list(si.on_update)
                    )
                    changed = True
                out.append(inst)
            if changed:
                while len(bb.instructions):
                    bb.instructions.pop()
                for i in out:
                    bb.instructions.append(i)
    return n


def bc(ap, reps):
    """Replace a trailing singleton dim with a zero-stride broadcast dim."""
    new = [list(d) for d in ap.ap]
    assert new[-1][1] == 1, new
    new[-1] = [0, reps]
    return bass.AP(tensor=ap.tensor, offset=ap.offset, ap=new)


def d1(ap):
    """Drop a trailing singleton dim."""
    new = [list(d) for d in ap.ap]
    assert new[-1][1] == 1, new
    return bass.AP(tensor=ap.tensor, offset=ap.offset, ap=new[:-1])


def abc(ap, reps):
    """Append a zero-stride broadcast dim."""
    new = [list(d) for d in ap.ap] + [[0, reps]]
    return bass.AP(tensor=ap.tensor, offset=ap.offset, ap=new)


def ibc(ap, pos, reps):
    """Insert a zero-stride broadcast dim at ap-list position pos."""
    new = [list(d) for d in ap.ap]
    new.insert(pos, [0, reps])
    return bass.AP(tensor=ap.tensor, offset=ap.offset, ap=new)


def build_kernel(repeat=1, timing=False):
    nc = bass.Bass("TRN2")
    # timing=True: inputs are internal (unbound, garbage) DRAM so a bench can
    # invoke the kernel without shipping 192 MB over the axon tunnel.
    kind = "Internal" if timing else "ExternalInput"
    pred = nc.dram_tensor("pred", [K_CORE, D], F32, kind=kind)
    targ = nc.dram_tensor("targ", [K_CORE, D], F32, kind=kind)
    NTR = NT * repeat
    out = nc.dram_tensor("out", [P, NTR * NGROUPS], F32, kind="ExternalOutput")

    # [NT, P, CPP*30] view: tile i, partition p holds CPP contiguous cells
    pred_v = pred.ap().rearrange("(n p c) d -> n p (c d)", n=NT, p=P, c=CPP)
    targ_v = targ.ap().rearrange("(n p c) d -> n p (c d)", n=NT, p=P, c=CPP)

    # scratch strip layout (per-cell values, 32 wide, bf16):
    #   [0:20]  obj-masked class diffs           (scale 1)
    #   [20:22] resp-masked contain diffs        (scale 1)
    #   [22:24] noo-masked conf diffs (ch 4, 9)  (scale sqrt(0.5))
    #   [24:28] resp-masked xy diffs             (scale sqrt(5))
    #   [28:32] resp-masked sqrt-wh diffs        (scale sqrt(5))
    SW = 32

    with tile.TileContext(nc) as tc:
        with (
            tc.tile_pool(name="io", bufs=3) as io,
            tc.tile_pool(name="mid", bufs=1) as mid,
            tc.tile_pool(name="strip", bufs=2) as strip,
            tc.tile_pool(name="accp", bufs=1) as accp,
        ):
            acc_all = accp.tile([P, NTR * NGROUPS], F32)
            eps_t = accp.tile([P, 1], F32)
            nc.vector.memset(eps_t[:], EPS)

            for rit in range(NTR):
                it = rit % NT
                pt = io.tile([P, CPP * D], BF16, tag="pt")
                tt = io.tile([P, CPP * D], BF16, tag="tt")
                # SWDGE cast-on-DMA: fp32 HBM -> bf16 SBUF
                nc.gpsimd.dma_start(out=pt[:], in_=pred_v[it])
                nc.gpsimd.dma_start(out=tt[:], in_=targ_v[it])

                p3 = pt[:].rearrange("p (c d) -> p c d", d=D)   # [128,CPP,30]
                t3 = tt[:].rearrange("p (c d) -> p c d", d=D)
                pb = pt[:].rearrange("p (c b f) -> p c b f", b=2 * 3, f=5)[:, :, 0:2, :]
                tb = tt[:].rearrange("p (c b f) -> p c b f", b=2 * 3, f=5)[:, :, 0:2, :]
                # pb/tb: [128, CPP, 2, 5] box view

                scratch = strip.tile([P, CPP, SW], BF16, tag="scratch")

                # ---- masks (exact {0,1} in bf16) ----
                obj = mid.tile([P, CPP, 1], BF16, tag="obj")
                noo = mid.tile([P, CPP, 1], BF16, tag="noo")
                t4 = t3[:, :, 4:5]
                nc.vector.tensor_scalar(out=obj[:], in0=t4, scalar1=0.0,
                                        scalar2=None, op0=Alu.is_gt)
                nc.vector.tensor_scalar(out=noo[:], in0=t4, scalar1=0.0,
                                        scalar2=None, op0=Alu.is_le)

                # ---- classes: (p-t)*obj into scratch[0:20] ----
                dcls = mid.tile([P, CPP, 20], BF16, tag="dcls")
                nc.vector.tensor_tensor(out=dcls[:], in0=p3[:, :, 10:30],
                                        in1=t3[:, :, 10:30], op=Alu.subtract)
                nc.vector.tensor_tensor(out=scratch[:, :, 0:20], in0=dcls[:],
                                        in1=bc(obj[:], 20), op=Alu.mult)

                # ---- noobj: (p-t)*noo on conf channels into scratch[22:24] ----
                d49 = mid.tile([P, CPP, 2], BF16, tag="d49")
                nc.vector.tensor_tensor(out=d49[:], in0=d1(pb[:, :, :, 4:5]),
                                        in1=d1(tb[:, :, :, 4:5]), op=Alu.subtract)
                nc.vector.tensor_tensor(out=scratch[:, :, 22:24], in0=d49[:],
                                        in1=bc(noo[:], 2), op=Alu.mult)

                # ---- xyxy for both tensors (packed [128, CPP, 2, 2] bf16) ----
                uvp = mid.tile([P, CPP, 2, 2], BF16, tag="uvp")
                uvt = mid.tile([P, CPP, 2, 2], BF16, tag="uvt")
                hwp = mid.tile([P, CPP, 2, 2], BF16, tag="hwp")
                hwt = mid.tile([P, CPP, 2, 2], BF16, tag="hwt")
                nc.scalar.mul(uvp[:], pb[:, :, :, 0:2], INV_S)
                nc.scalar.mul(uvt[:], tb[:, :, :, 0:2], INV_S)
                nc.scalar.mul(hwp[:], pb[:, :, :, 2:4], 0.5)
                nc.scalar.mul(hwt[:], tb[:, :, :, 2:4], 0.5)

                xy1p = mid.tile([P, CPP, 2, 2], BF16, tag="xy1p")
                xy2p = mid.tile([P, CPP, 2, 2], BF16, tag="xy2p")
                xy1t = mid.tile([P, CPP, 2, 2], BF16, tag="xy1t")
                xy2t = mid.tile([P, CPP, 2, 2], BF16, tag="xy2t")
                nc.vector.tensor_tensor(out=xy1p[:], in0=uvp[:], in1=hwp[:], op=Alu.subtract)
                nc.vector.tensor_tensor(out=xy2p[:], in0=uvp[:], in1=hwp[:], op=Alu.add)
                nc.vector.tensor_tensor(out=xy1t[:], in0=uvt[:], in1=hwt[:], op=Alu.subtract)
                nc.vector.tensor_tensor(out=xy2t[:], in0=uvt[:], in1=hwt[:], op=Alu.add)

                # areas (reference-exact: from xyxy differences)
                dxyp = mid.tile([P, CPP, 2, 2], BF16, tag="dxyp")
                dxyt = mid.tile([P, CPP, 2, 2], BF16, tag="dxyt")
                nc.vector.tensor_tensor(out=dxyp[:], in0=xy2p[:], in1=xy1p[:], op=Alu.subtract)
                nc.vector.tensor_tensor(out=dxyt[:], in0=xy2t[:], in1=xy1t[:], op=Alu.subtract)
                areap = mid.tile([P, CPP, 2], BF16, tag="areap")
                areat = mid.tile([P, CPP, 2], BF16, tag="areat")
                nc.vector.tensor_tensor(out=areap[:], in0=d1(dxyp[:, :, :, 0:1]),
                                        in1=d1(dxyp[:, :, :, 1:2]), op=Alu.mult)
                nc.vector.tensor_tensor(out=areat[:], in0=d1(dxyt[:, :, :, 0:1]),
                                        in1=d1(dxyt[:, :, :, 1:2]), op=Alu.mult)

                # ---- pairwise IoU; loop over target box j, pred boxes vectorized ----
                inter_a = mid.tile([P, CPP, 2, 2], F32, tag="inter")  # [.., j, i]
                union_a = mid.tile([P, CPP, 2, 2], F32, tag="union")
                for j in range(2):
                    xy1tj = ibc(xy1t[:, :, j, :], 2, 2)   # [P, CPP, 2(i), 2(ch)]
                    xy2tj = ibc(xy2t[:, :, j, :], 2, 2)
                    lt = mid.tile([P, CPP, 2, 2], BF16, tag="lt")
                    rb = mid.tile([P, CPP, 2, 2], BF16, tag="rb")
                    nc.vector.tensor_tensor(out=lt[:], in0=xy1p[:], in1=xy1tj, op=Alu.max)
                    nc.vector.tensor_tensor(out=rb[:], in0=xy2p[:], in1=xy2tj, op=Alu.min)
                    clip = mid.tile([P, CPP, 2, 2], BF16, tag="clip")
                    nc.vector.scalar_tensor_tensor(out=clip[:], in0=rb[:], scalar=0.0,
                                                   in1=lt[:], op0=Alu.bypass,
                                                   op1=Alu.subtract)
                    nc.vector.tensor_scalar(out=clip[:], in0=clip[:], scalar1=0.0,
                                            scalar2=None, op0=Alu.max)
                    nc.vector.tensor_tensor(out=inter_a[:, :, j, :],
                                            in0=d1(clip[:, :, :, 0:1]),
                                            in1=d1(clip[:, :, :, 1:2]), op=Alu.mult)
                    usum = mid.tile([P, CPP, 2], F32, tag="usum")
                    nc.vector.tensor_tensor(out=usum[:], in0=areap[:],
                                            in1=bc(areat[:, :, j:j + 1], 2), op=Alu.add)
                    nc.vector.tensor_tensor(out=union_a[:, :, j, :], in0=usum[:],
                                            in1=inter_a[:, :, j, :], op=Alu.subtract)
                rec = mid.tile([P, CPP, 2, 2], F32, tag="rec")
                nc.vector.reciprocal(out=rec[:].rearrange("p c j i -> p (c j i)"),
                                     in_=union_a[:].rearrange("p c j i -> p (c j i)"))
                iou = mid.tile([P, CPP, 2, 2], BF16, tag="iou")
                nc.vector.tensor_tensor(out=iou[:], in0=inter_a[:], in1=rec[:], op=Alu.mult)

                # ---- argmax over pred axis i, per target j ----
                g = mid.tile([P, CPP, 2], BF16, tag="g")    # 1.0 if pred box 1 wins
                m = mid.tile([P, CPP, 2], BF16, tag="m")    # max iou
                nc.vector.tensor_tensor(out=g[:], in0=d1(iou[:, :, :, 1:2]),
                                        in1=d1(iou[:, :, :, 0:1]), op=Alu.is_gt)
                nc.vector.tensor_tensor(out=m[:], in0=d1(iou[:, :, :, 1:2]),
                                        in1=d1(iou[:, :, :, 0:1]), op=Alu.max)

                # ---- conf targets (last-write-wins) ----
                m0, m1 = m[:, :, 0:1], m[:, :, 1:2]
                g0, g1 = g[:, :, 0:1], g[:, :, 1:2]
                dm = mid.tile([P, CPP, 1], BF16, tag="dm")
                gdm = mid.tile([P, CPP, 1], BF16, tag="gdm")
                ct = mid.tile([P, CPP, 2], BF16, tag="ct")
                nc.vector.tensor_tensor(out=dm[:], in0=m0, in1=m1, op=Alu.subtract)
                nc.vector.tensor_tensor(out=gdm[:], in0=g1, in1=dm[:], op=Alu.mult)
                nc.vector.tensor_tensor(out=ct[:, :, 0:1], in0=m1, in1=gdm[:], op=Alu.add)
                nc.vector.tensor_tensor(out=ct[:, :, 1:2], in0=m0, in1=gdm[:], op=Alu.subtract)

                # ---- responsibility masks (exact {0,1}) ----
                gg = mid.tile([P, CPP, 1], BF16, tag="gg")
                s01 = mid.tile([P, CPP, 1], BF16, tag="s01")
                rr = mid.tile([P, CPP, 2], BF16, tag="rr")
                nc.vector.tensor_tensor(out=gg[:], in0=g0, in1=g1, op=Alu.mult)
                nc.vector.tensor_tensor(out=s01[:], in0=g0, in1=g1, op=Alu.add)
                nc.vector.tensor_scalar(out=rr[:, :, 0:1], in0=gg[:], scalar1=-1.0,
                                        scalar2=1.0, op0=Alu.mult, op1=Alu.add)
                nc.vector.scalar_tensor_tensor(out=rr[:, :, 1:2], in0=gg[:], scalar=-1.0,
                                               in1=s01[:], op0=Alu.mult, op1=Alu.add)
                rm = mid.tile([P, CPP, 2], BF16, tag="rm")
                nc.vector.tensor_tensor(out=rm[:], in0=rr[:], in1=bc(obj[:], 2), op=Alu.mult)

                # ---- contain: (pconf - ct)*rm into scratch[20:22] ----
                e = mid.tile([P, CPP, 2], BF16, tag="e")
                nc.vector.tensor_tensor(out=e[:], in0=d1(pb[:, :, :, 4:5]), in1=ct[:],
                                        op=Alu.subtract)
                nc.vector.tensor_tensor(out=scratch[:, :, 20:22], in0=e[:], in1=rm[:],
                                        op=Alu.mult)

                # ---- loc xy: (pxy - txy)*rm into scratch[24:28] ----
                dxy = mid.tile([P, CPP, 2, 2], BF16, tag="dxy")
                nc.vector.tensor_tensor(out=dxy[:], in0=pb[:, :, :, 0:2],
                                        in1=tb[:, :, :, 0:2], op=Alu.subtract)
                sxy = scratch[:, :, 24:28].rearrange("p c (b f) -> p c b f", b=2)
                nc.vector.tensor_tensor(out=sxy, in0=dxy[:], in1=abc(rm[:], 2), op=Alu.mult)

                # ---- loc wh: (sqrt(pwh+eps)-sqrt(twh+eps))*rm into scratch[28:32] ----
                sqp = mid.tile([P, CPP, 2, 2], BF16, tag="sqp")
                sqt = mid.tile([P, CPP, 2, 2], BF16, tag="sqt")
                nc.scalar.activation(out=sqp[:], in_=pb[:, :, :, 2:4], func=Act.Sqrt,
                                     bias=eps_t[:], scale=1.0)
                nc.scalar.activation(out=sqt[:], in_=tb[:, :, :, 2:4], func=Act.Sqrt,
                                     bias=eps_t[:], scale=1.0)
                dwh = mid.tile([P, CPP, 2, 2], BF16, tag="dwh")
                nc.vector.tensor_tensor(out=dwh[:], in0=sqp[:], in1=sqt[:], op=Alu.subtract)
                swh = scratch[:, :, 28:32].rearrange("p c (b f) -> p c b f", b=2)
                nc.vector.tensor_tensor(out=swh, in0=dwh[:], in1=abc(rm[:], 2), op=Alu.mult)

                # ---- fused square+sum per scale group (in place) ----
                seg1 = scratch[:, :, 0:22]
                seg2 = scratch[:, :, 22:24]
                seg3 = scratch[:, :, 24:32]
                base = rit * NGROUPS
                nc.scalar.activation(out=seg1, in_=seg1, func=Act.Square, scale=1.0,
                                     accum_out=acc_all[:, base:base + 1])
                nc.scalar.activation(out=seg2, in_=seg2, func=Act.Square, scale=SQRT_HALF,
                                     accum_out=acc_all[:, base + 1:base + 2])
                nc.scalar.activation(out=seg3, in_=seg3, func=Act.Square, scale=SQRT5,
                                     accum_out=acc_all[:, base + 2:base + 3])

            nc.sync.dma_start(out=out[:], in_=acc_all[:])

    split_sync_waits(nc)
    return nc


_NC_CACHE = None


def kernel(pred_tensor: np.ndarray, target_tensor: np.ndarray) -> np.ndarray:
    global _NC_CACHE
    if _NC_CACHE is None:
        _NC_CACHE = build_kernel()
    nc = _NC_CACHE

    p = np.ascontiguousarray(pred_tensor, dtype=np.float32).reshape(N_CORES, K_CORE, D)
    t = np.ascontiguousarray(target_tensor, dtype=np.float32).reshape(N_CORES, K_CORE, D)
    in_maps = [{"pred": p[i], "targ": t[i]} for i in range(N_CORES)]
    res = run_bass_kernel_spmd(nc, in_maps, core_ids=list(range(N_CORES)))
    total = 0.0
    for i in range(N_CORES):
        total += res.results[i]["out"].astype(np.float64).sum()
    return np.float32(total / BATCH)


# revision 17
# speedup vs baseline: 4.5800x; 4.5800x over previous
"""YOLO-style loss kernel for Trainium2, 8-core data-parallel.

Strategy: shard the 16384 batch across 8 cores (2048 each = 100352 grid
cells). Each core streams its [cells, 30] fp32 pred/target arrays through
SBUF in 4 wide tiles (HWDGE DMA at ~360 GB/s), computes every loss term as a
mask-in-{0,1} times value, concatenates the masked values into one scratch
strip per tile, and reduces with the scalar engine's
ACTIVATE(Square, scale=sqrt(term_weight), accum_out=...) — three accum ops
per tile, one per loss weight (1, 0.5, 5), so term weights stay exact fp32
while masks are exact binary. The host sums the 8x[128, NT*3] partials and
divides by N.

Per-cell math (channels [x0,y0,w0,h0,c0, x1,y1,w1,h1,c1, 20 class]):
  obj  = t4 > 0, noo = t4 == 0, d = p - t
  noobj term   = 0.5*noo*(d4^2 + d9^2)
  class term   = obj * sum_cls d^2
  iou(i,j) of pred box i vs target box j from xyxy at x/7 +- w/2
  g_j = iou(1,j) > iou(0,j) (argmax), m_j = max_i iou(i,j)
  conf targets c0 = m1 + g1*(m0-m1), c1 = m0 - g1*(m0-m1) (last-write-wins)
  resp_0 = obj*(1-g0*g1), resp_1 = obj*(g0+g1-g0*g1)
  contain term = resp_b*(pconf_b - c_b)^2
  loc term     = 5*resp_b*(dxy^2 + (sqrt(pwh+eps)-sqrt(twh+eps))^2)
"""

import math

import numpy as np
import concourse.bass as bass
import concourse.tile as tile
from concourse import mybir
from concourse.bass_utils import run_bass_kernel_spmd

F32 = mybir.dt.float32
BF16 = mybir.dt.bfloat16
Alu = mybir.AluOpType
Act = mybir.ActivationFunctionType

# problem constants (hardcoded per harness contract)
BATCH = 16384
S = 7
D = 30
N_CORES = 8
B_PER = BATCH // N_CORES            # 2048
K_CORE = B_PER * S * S              # 100352 cells/core
P = 128
CELLS_PER_PART = K_CORE // P        # 784
NT = 4                              # tiles per core
CPP = CELLS_PER_PART // NT          # cells per partition per tile
INV_S = 1.0 / 7.0
EPS = 1e-6
SQRT5 = math.sqrt(5.0)
SQRT_HALF = math.sqrt(0.5)
NGROUPS = 3                         # accum scale groups: 1.0 / sqrt(.5) / sqrt(5)


def split_sync_waits(nc, max_attached=1):
    """This container's walrus build rejects >1 semaphore wait attached to an
    instruction. Hoist the extras into standalone EventSemaphore wait
    instructions (what raw-bass wait_ge emits), which it accepts."""
    n = 0
    for func in nc.m.functions:
        for bb in func.blocks:
            insts = list(bb.instructions)
            out = []
            changed = False
            for inst in insts:
                si = inst.sync_info
                if si is not None and len(si.on_wait) > max_attached:
                    waits = list(si.on_wait)
                    keep, hoist = waits[:max_attached], waits[max_attached:]
                    for k, w in enumerate(hoist):
                        wi = mybir.InstEventSemaphore(
                            name=f"{inst.name}-hw{k}", ins=[], outs=[]
                        )
                        wi.engine = inst.engine
                        wi.sync_info = mybir.SyncInfo(on_wait=[w], on_update=[])
                        nc.register_instruction(wi, overwrite=True)
                        out.append(wi)
                        n += 1
                    inst.sync_info = mybir.SyncInfo(
                        on_wait=keep, on_update=list(si.on_update)
                    )
                    changed = True
                out.append(inst)
            if changed:
                while len(bb.instructions):
                    bb.instructions.pop()
                for i in out:
                    bb.instructions.append(i)
    return n


def bc(ap, reps):
    """Replace a trailing singleton dim with a zero-stride broadcast dim."""
    new = [list(d) for d in ap.ap]
    assert new[-1][1] == 1, new
    new[-1] = [0, reps]
    return bass.AP(tensor=ap.tensor, offset=ap.offset, ap=new)


def d1(ap):
    """Drop a trailing singleton dim."""
    new = [list(d) for d in ap.ap]
    assert new[-1][1] == 1, new
    return bass.AP(tensor=ap.tensor, offset=ap.offset, ap=new[:-1])


def abc(ap, reps):
    """Append a zero-stride broadcast dim."""
    new = [list(d) for d in ap.ap] + [[0, reps]]
    return bass.AP(tensor=ap.tensor, offset=ap.offset, ap=new)


def ibc(ap, pos, reps):
    """Insert a zero-stride broadcast dim at ap-list position pos."""
    new = [list(d) for d in ap.ap]
    new.insert(pos, [0, reps])
    return bass.AP(tensor=ap.tensor, offset=ap.offset, ap=new)


def build_kernel(repeat=1, timing=False):
    nc = bass.Bass("TRN2")
    # timing=True: inputs are internal (unbound, garbage) DRAM so a bench can
    # invoke the kernel without shipping 192 MB over the axon tunnel.
    kind = "Internal" if timing else "ExternalInput"
    pred = nc.dram_tensor("pred", [K_CORE, D], F32, kind=kind)
    targ = nc.dram_tensor("targ", [K_CORE, D], F32, kind=kind)
    NTR = NT * repeat
    out = nc.dram_tensor("out", [P, NTR * NGROUPS], F32, kind="ExternalOutput")

    # [NT, P, CPP*30] view: tile i, partition p holds CPP contiguous cells
    pred_v = pred.ap().rearrange("(n p c) d -> n p (c d)", n=NT, p=P, c=CPP)
    targ_v = targ.ap().rearrange("(n p c) d -> n p (c d)", n=NT, p=P, c=CPP)

    # scratch strip layout (per-cell values, 32 wide, bf16):
    #   [0:20]  obj-masked class diffs           (scale 1)
    #   [20:22] resp-masked contain diffs        (scale 1)
    #   [22:24] noo-masked conf diffs (ch 4, 9)  (scale sqrt(0.5))
    #   [24:28] resp-masked xy diffs             (scale sqrt(5))
    #   [28:32] resp-masked sqrt-wh diffs        (scale sqrt(5))
    SW = 32

    with tile.TileContext(nc) as tc:
        with (
            tc.tile_pool(name="io", bufs=2) as io,
            tc.tile_pool(name="mid", bufs=1) as mid,
            tc.tile_pool(name="strip", bufs=2) as strip,
            tc.tile_pool(name="accp", bufs=1) as accp,
        ):
            acc_all = accp.tile([P, NTR * NGROUPS], F32)
            eps_t = accp.tile([P, 1], F32)
            nc.vector.memset(eps_t[:], EPS)

            for rit in range(NTR):
                it = rit % NT
                pt = io.tile([P, CPP * D], F32, tag="pt")
                tt = io.tile([P, CPP * D], F32, tag="tt")
                nc.sync.dma_start(out=pt[:], in_=pred_v[it])
                nc.sync.dma_start(out=tt[:], in_=targ_v[it])

                p3 = pt[:].rearrange("p (c d) -> p c d", d=D)   # [128,CPP,30]
                t3 = tt[:].rearrange("p (c d) -> p c d", d=D)
                pb = pt[:].rearrange("p (c b f) -> p c b f", b=2 * 3, f=5)[:, :, 0:2, :]
                tb = tt[:].rearrange("p (c b f) -> p c b f", b=2 * 3, f=5)[:, :, 0:2, :]
                # pb/tb: [128, CPP, 2, 5] box view

                scratch = strip.tile([P, CPP, SW], BF16, tag="scratch")

                # ---- masks (exact {0,1} in bf16) ----
                obj = mid.tile([P, CPP, 1], BF16, tag="obj")
                noo = mid.tile([P, CPP, 1], BF16, tag="noo")
                t4 = t3[:, :, 4:5]
                nc.vector.tensor_scalar(out=obj[:], in0=t4, scalar1=0.0,
                                        scalar2=None, op0=Alu.is_gt)
                nc.vector.tensor_scalar(out=noo[:], in0=t4, scalar1=0.0,
                                        scalar2=None, op0=Alu.is_le)

                # ---- classes: (p-t)*obj into scratch[0:20] ----
                dcls = mid.tile([P, CPP, 20], BF16, tag="dcls")
                nc.vector.tensor_tensor(out=dcls[:], in0=p3[:, :, 10:30],
                                        in1=t3[:, :, 10:30], op=Alu.subtract)
                nc.vector.tensor_tensor(out=scratch[:, :, 0:20], in0=dcls[:],
                                        in1=bc(obj[:], 20), op=Alu.mult)

                # ---- noobj: (p-t)*noo on conf channels into scratch[22:24] ----
                d49 = mid.tile([P, CPP, 2], BF16, tag="d49")
                nc.vector.tensor_tensor(out=d49[:], in0=d1(pb[:, :, :, 4:5]),
                                        in1=d1(tb[:, :, :, 4:5]), op=Alu.subtract)
                nc.vector.tensor_tensor(out=scratch[:, :, 22:24], in0=d49[:],
                                        in1=bc(noo[:], 2), op=Alu.mult)

                # ---- xyxy for both tensors (packed [128, CPP, 2, 2] bf16) ----
                uvp = mid.tile([P, CPP, 2, 2], BF16, tag="uvp")
                uvt = mid.tile([P, CPP, 2, 2], BF16, tag="uvt")
                hwp = mid.tile([P, CPP, 2, 2], BF16, tag="hwp")
                hwt = mid.tile([P, CPP, 2, 2], BF16, tag="hwt")
                nc.scalar.mul(uvp[:], pb[:, :, :, 0:2], INV_S)
                nc.scalar.mul(uvt[:], tb[:, :, :, 0:2], INV_S)
                nc.scalar.mul(hwp[:], pb[:, :, :, 2:4], 0.5)
                nc.scalar.mul(hwt[:], tb[:, :, :, 2:4], 0.5)

                xy1p = mid.tile([P, CPP, 2, 2], BF16, tag="xy1p")
                xy2p = mid.tile([P, CPP, 2, 2], BF16, tag="xy2p")
                xy1t = mid.tile([P, CPP, 2, 2], BF16, tag="xy1t")
                xy2t = mid.tile([P, CPP, 2, 2], BF16, tag="xy2t")
                nc.vector.tensor_tensor(out=xy1p[:], in0=uvp[:], in1=hwp[:], op=Alu.subtract)
                nc.vector.tensor_tensor(out=xy2p[:], in0=uvp[:], in1=hwp[:], op=Alu.add)
                nc.vector.tensor_tensor(out=xy1t[:], in0=uvt[:], in1=hwt[:], op=Alu.subtract)
                nc.vector.tensor_tensor(out=xy2t[:], in0=uvt[:], in1=hwt[:], op=Alu.add)

                # areas (reference-exact: from xyxy differences)
                dxyp = mid.tile([P, CPP, 2, 2], BF16, tag="dxyp")
                dxyt = mid.tile([P, CPP, 2, 2], BF16, tag="dxyt")
                nc.vector.tensor_tensor(out=dxyp[:], in0=xy2p[:], in1=xy1p[:], op=Alu.subtract)
                nc.vector.tensor_tensor(out=dxyt[:], in0=xy2t[:], in1=xy1t[:], op=Alu.subtract)
                areap = mid.tile([P, CPP, 2], BF16, tag="areap")
                areat = mid.tile([P, CPP, 2], BF16, tag="areat")
                nc.vector.tensor_tensor(out=areap[:], in0=d1(dxyp[:, :, :, 0:1]),
                                        in1=d1(dxyp[:, :, :, 1:2]), op=Alu.mult)
                nc.vector.tensor_tensor(out=areat[:], in0=d1(dxyt[:, :, :, 0:1]),
                                        in1=d1(dxyt[:, :, :, 1:2]), op=Alu.mult)

                # ---- pairwise IoU; loop over target box j, pred boxes vectorized ----
                inter_a = mid.tile([P, CPP, 2, 2], F32, tag="inter")  # [.., j, i]
                union_a = mid.tile([P, CPP, 2, 2], F32, tag="union")
                for j in range(2):
                    xy1tj = ibc(xy1t[:, :, j, :], 2, 2)   # [P, CPP, 2(i), 2(ch)]
                    xy2tj = ibc(xy2t[:, :, j, :], 2, 2)
                    lt = mid.tile([P, CPP, 2, 2], BF16, tag="lt")
                    rb = mid.tile([P, CPP, 2, 2], BF16, tag="rb")
                    nc.vector.tensor_tensor(out=lt[:], in0=xy1p[:], in1=xy1tj, op=Alu.max)
                    nc.vector.tensor_tensor(out=rb[:], in0=xy2p[:], in1=xy2tj, op=Alu.min)
                    clip = mid.tile([P, CPP, 2, 2], BF16, tag="clip")
                    nc.vector.scalar_tensor_tensor(out=clip[:], in0=rb[:], scalar=0.0,
                                                   in1=lt[:], op0=Alu.bypass,
                                                   op1=Alu.subtract)
                    nc.vector.tensor_scalar(out=clip[:], in0=clip[:], scalar1=0.0,
                                            scalar2=None, op0=Alu.max)
                    nc.vector.tensor_tensor(out=inter_a[:, :, j, :],
                                            in0=d1(clip[:, :, :, 0:1]),
                                            in1=d1(clip[:, :, :, 1:2]), op=Alu.mult)
                    usum = mid.tile([P, CPP, 2], F32, tag="usum")
                    nc.vector.tensor_tensor(out=usum[:], in0=areap[:],
                                            in1=bc(areat[:, :, j:j + 1], 2), op=Alu.add)
                    nc.vector.tensor_tensor(out=union_a[:, :, j, :], in0=usum[:],
                                            in1=inter_a[:, :, j, :], op=Alu.subtract)
                rec = mid.tile([P, CPP, 2, 2], F32, tag="rec")
                nc.vector.reciprocal(out=rec[:].rearrange("p c j i -> p (c j i)"),
                                     in_=union_a[:].rearrange("p c j i -> p (c j i)"))
                iou = mid.tile([P, CPP, 2, 2], BF16, tag="iou")
                nc.vector.tensor_tensor(out=iou[:], in0=inter_a[:], in1=rec[:], op=Alu.mult)

                # ---- argmax over pred axis i, per target j ----
                g = mid.tile([P, CPP, 2], BF16, tag="g")    # 1.0 if pred box 1 wins
                m = mid.tile([P, CPP, 2], BF16, tag="m")    # max iou
                nc.vector.tensor_tensor(out=g[:], in0=d1(iou[:, :, :, 1:2]),
                                        in1=d1(iou[:, :, :, 0:1]), op=Alu.is_gt)
                nc.vector.tensor_tensor(out=m[:], in0=d1(iou[:, :, :, 1:2]),
                                        in1=d1(iou[:, :, :, 0:1]), op=Alu.max)

                # ---- conf targets (last-write-wins) ----
                m0, m1 = m[:, :, 0:1], m[:, :, 1:2]
                g0, g1 = g[:, :, 0:1], g[:, :, 1:2]
                dm = mid.tile([P, CPP, 1], BF16, tag="dm")
                gdm = mid.tile([P, CPP, 1], BF16, tag="gdm")
                ct = mid.tile([P, CPP, 2], BF16, tag="ct")
                nc.vector.tensor_tensor(out=dm[:], in0=m0, in1=m1, op=Alu.subtract)
                nc.vector.tensor_tensor(out=gdm[:], in0=g1, in1=dm[:], op=Alu.mult)
                nc.vector.tensor_tensor(out=ct[:, :, 0:1], in0=m1, in1=gdm[:], op=Alu.add)
                nc.vector.tensor_tensor(out=ct[:, :, 1:2], in0=m0, in1=gdm[:], op=Alu.subtract)

                # ---- responsibility masks (exact {0,1}) ----
                gg = mid.tile([P, CPP, 1], BF16, tag="gg")
                s01 = mid.tile([P, CPP, 1], BF16, tag="s01")
                rr = mid.tile([P, CPP, 2], BF16, tag="rr")
                nc.vector.tensor_tensor(out=gg[:], in0=g0, in1=g1, op=Alu.mult)
                nc.vector.tensor_tensor(out=s01[:], in0=g0, in1=g1, op=Alu.add)
                nc.vector.tensor_scalar(out=rr[:, :, 0:1], in0=gg[:], scalar1=-1.0,
                                        scalar2=1.0, op0=Alu.mult, op1=Alu.add)
                nc.vector.scalar_tensor_tensor(out=rr[:, :, 1:2], in0=gg[:], scalar=-1.0,
                                               in1=s01[:], op0=Alu.mult, op1=Alu.add)
                rm = mid.tile([P, CPP, 2], BF16, tag="rm")
                nc.vector.tensor_tensor(out=rm[:], in0=rr[:], in1=bc(obj[:], 2), op=Alu.mult)

                # ---- contain: (pconf - ct)*rm into scratch[20:22] ----
                pc2 = mid.tile([P, CPP, 2], BF16, tag="pc2")
                nc.vector.tensor_scalar_mul(pc2[:], d1(pb[:, :, :, 4:5]), 1.0)
                e = mid.tile([P, CPP, 2], BF16, tag="e")
                nc.vector.tensor_tensor(out=e[:], in0=pc2[:], in1=ct[:],
                                        op=Alu.subtract)
                nc.vector.tensor_tensor(out=scratch[:, :, 20:22], in0=e[:], in1=rm[:],
                                        op=Alu.mult)

                # ---- loc xy: (pxy - txy)*rm into scratch[24:28] ----
                dxy = mid.tile([P, CPP, 2, 2], BF16, tag="dxy")
                nc.vector.tensor_tensor(out=dxy[:], in0=pb[:, :, :, 0:2],
                                        in1=tb[:, :, :, 0:2], op=Alu.subtract)
                sxy = scratch[:, :, 24:28].rearrange("p c (b f) -> p c b f", b=2)
                nc.vector.tensor_tensor(out=sxy, in0=dxy[:], in1=abc(rm[:], 2), op=Alu.mult)

                # ---- loc wh: (sqrt(pwh+eps)-sqrt(twh+eps))*rm into scratch[28:32] ----
                sqp = mid.tile([P, CPP, 2, 2], BF16, tag="sqp")
                sqt = mid.tile([P, CPP, 2, 2], BF16, tag="sqt")
                nc.scalar.activation(out=sqp[:], in_=pb[:, :, :, 2:4], func=Act.Sqrt,
                                     bias=eps_t[:], scale=1.0)
                nc.scalar.activation(out=sqt[:], in_=tb[:, :, :, 2:4], func=Act.Sqrt,
                                     bias=eps_t[:], scale=1.0)
                dwh = mid.tile([P, CPP, 2, 2], BF16, tag="dwh")
                nc.vector.tensor_tensor(out=dwh[:], in0=sqp[:], in1=sqt[:], op=Alu.subtract)
                swh = scratch[:, :, 28:32].rearrange("p c (b f) -> p c b f", b=2)
                nc.vector.tensor_tensor(out=swh, in0=dwh[:], in1=abc(rm[:], 2), op=Alu.mult)

                # ---- fused square+sum per scale group (in place) ----
                seg1 = scratch[:, :, 0:22]
                seg2 = scratch[:, :, 22:24]
                seg3 = scratch[:, :, 24:32]
                base = rit * NGROUPS
                nc.scalar.activation(out=seg1, in_=seg1, func=Act.Square, scale=1.0,
                                     accum_out=acc_all[:, base:base + 1])
                nc.scalar.activation(out=seg2, in_=seg2, func=Act.Square, scale=SQRT_HALF,
                                     accum_out=acc_all[:, base + 1:base + 2])
                nc.scalar.activation(out=seg3, in_=seg3, func=Act.Square, scale=SQRT5,
                                     accum_out=acc_all[:, base + 2:base + 3])

            nc.sync.dma_start(out=out[:], in_=acc_all[:])

    split_sync_waits(nc)
    return nc


_NC_CACHE = None


def kernel(pred_tensor: np.ndarray, target_tensor: np.ndarray) -> np.ndarray:
    global _NC_CACHE
    if _NC_CACHE is None:
        _NC_CACHE = build_kernel()
    nc = _NC_CACHE

    p = np.ascontiguousarray(pred_tensor, dtype=np.float32).reshape(N_CORES, K_CORE, D)
    t = np.ascontiguousarray(target_tensor, dtype=np.float32).reshape(N_CORES, K_CORE, D)
    in_maps = [{"pred": p[i], "targ": t[i]} for i in range(N_CORES)]
    res = run_bass_kernel_spmd(nc, in_maps, core_ids=list(range(N_CORES)))
    total = 0.0
    for i in range(N_CORES):
        total += res.results[i]["out"].astype(np.float64).sum()
    return np.float32(total / BATCH)


# revision 18
# speedup vs baseline: 7.2109x; 1.5745x over previous
"""YOLO-style loss kernel for Trainium2, 8-core data-parallel.

Strategy: shard the 16384 batch across 8 cores (2048 each = 100352 grid
cells). Each core streams its [cells, 30] fp32 pred/target arrays through
SBUF in 4 wide tiles (HWDGE DMA at ~360 GB/s), computes every loss term as a
mask-in-{0,1} times value, concatenates the masked values into one scratch
strip per tile, and reduces with the scalar engine's
ACTIVATE(Square, scale=sqrt(term_weight), accum_out=...) — three accum ops
per tile, one per loss weight (1, 0.5, 5), so term weights stay exact fp32
while masks are exact binary. The host sums the 8x[128, NT*3] partials and
divides by N.

Per-cell math (channels [x0,y0,w0,h0,c0, x1,y1,w1,h1,c1, 20 class]):
  obj  = t4 > 0, noo = t4 == 0, d = p - t
  noobj term   = 0.5*noo*(d4^2 + d9^2)
  class term   = obj * sum_cls d^2
  iou(i,j) of pred box i vs target box j from xyxy at x/7 +- w/2
  g_j = iou(1,j) > iou(0,j) (argmax), m_j = max_i iou(i,j)
  conf targets c0 = m1 + g1*(m0-m1), c1 = m0 - g1*(m0-m1) (last-write-wins)
  resp_0 = obj*(1-g0*g1), resp_1 = obj*(g0+g1-g0*g1)
  contain term = resp_b*(pconf_b - c_b)^2
  loc term     = 5*resp_b*(dxy^2 + (sqrt(pwh+eps)-sqrt(twh+eps))^2)
"""

import math

import numpy as np
import concourse.bass as bass
import concourse.tile as tile
from concourse import mybir
from concourse.bass_utils import run_bass_kernel_spmd

F32 = mybir.dt.float32
BF16 = mybir.dt.bfloat16
Alu = mybir.AluOpType
Act = mybir.ActivationFunctionType

# problem constants (hardcoded per harness contract)
BATCH = 16384
S = 7
D = 30
N_CORES = 8
B_PER = BATCH // N_CORES            # 2048
K_CORE = B_PER * S * S              # 100352 cells/core
P = 128
CELLS_PER_PART = K_CORE // P        # 784
NT = 4                              # tiles per core
CPP = CELLS_PER_PART // NT          # cells per partition per tile
INV_S = 1.0 / 7.0
EPS = 1e-6
SQRT5 = math.sqrt(5.0)
SQRT_HALF = math.sqrt(0.5)
NGROUPS = 3                         # accum scale groups: 1.0 / sqrt(.5) / sqrt(5)


def split_sync_waits(nc, max_attached=1):
    """This container's walrus build rejects >1 semaphore wait attached to an
    instruction. Hoist the extras into standalone EventSemaphore wait
    instructions (what raw-bass wait_ge emits), which it accepts."""
    n = 0
    for func in nc.m.functions:
        for bb in func.blocks:
            insts = list(bb.instructions)
            out = []
            changed = False
            for inst in insts:
                si = inst.sync_info
                if si is not None and len(si.on_wait) > max_attached:
                    waits = list(si.on_wait)
                    keep, hoist = waits[:max_attached], waits[max_attached:]
                    for k, w in enumerate(hoist):
                        wi = mybir.InstEventSemaphore(
                            name=f"{inst.name}-hw{k}", ins=[], outs=[]
                        )
                        wi.engine = inst.engine
                        wi.sync_info = mybir.SyncInfo(on_wait=[w], on_update=[])
                        nc.register_instruction(wi, overwrite=True)
                        out.append(wi)
                        n += 1
                    inst.sync_info = mybir.SyncInfo(
                        on_wait=keep, on_update=list(si.on_update)
                    )
                    changed = True
                out.append(inst)
            if changed:
                while len(bb.instructions):
                    bb.instructions.pop()
                for i in out:
                    bb.instructions.append(i)
    return n


def bc(ap, reps):
    """Replace a trailing singleton dim with a zero-stride broadcast dim."""
    new = [list(d) for d in ap.ap]
    assert new[-1][1] == 1, new
    new[-1] = [0, reps]
    return bass.AP(tensor=ap.tensor, offset=ap.offset, ap=new)


def d1(ap):
    """Drop a trailing singleton dim."""
    new = [list(d) for d in ap.ap]
    assert new[-1][1] == 1, new
    return bass.AP(tensor=ap.tensor, offset=ap.offset, ap=new[:-1])


def abc(ap, reps):
    """Append a zero-stride broadcast dim."""
    new = [list(d) for d in ap.ap] + [[0, reps]]
    return bass.AP(tensor=ap.tensor, offset=ap.offset, ap=new)


def ibc(ap, pos, reps):
    """Insert a zero-stride broadcast dim at ap-list position pos."""
    new = [list(d) for d in ap.ap]
    new.insert(pos, [0, reps])
    return bass.AP(tensor=ap.tensor, offset=ap.offset, ap=new)


def build_kernel(repeat=1, timing=False):
    nc = bass.Bass("TRN2")
    # timing=True: inputs are internal (unbound, garbage) DRAM so a bench can
    # invoke the kernel without shipping 192 MB over the axon tunnel.
    kind = "Internal" if timing else "ExternalInput"
    pred = nc.dram_tensor("pred", [K_CORE, D], F32, kind=kind)
    targ = nc.dram_tensor("targ", [K_CORE, D], F32, kind=kind)
    NTR = NT * repeat
    out = nc.dram_tensor("out", [P, NTR * NGROUPS], F32, kind="ExternalOutput")

    # [NT, P, CPP*30] view: tile i, partition p holds CPP contiguous cells
    pred_v = pred.ap().rearrange("(n p c) d -> n p (c d)", n=NT, p=P, c=CPP)
    targ_v = targ.ap().rearrange("(n p c) d -> n p (c d)", n=NT, p=P, c=CPP)

    # scratch strip layout (per-cell values, 32 wide, bf16):
    #   [0:20]  obj-masked class diffs           (scale 1)
    #   [20:22] resp-masked contain diffs        (scale 1)
    #   [22:24] noo-masked conf diffs (ch 4, 9)  (scale sqrt(0.5))
    #   [24:28] resp-masked xy diffs             (scale sqrt(5))
    #   [28:32] resp-masked sqrt-wh diffs        (scale sqrt(5))
    SW = 32

    with tile.TileContext(nc) as tc:
        with (
            tc.tile_pool(name="io", bufs=2) as io,
            tc.tile_pool(name="mid", bufs=1) as mid,
            tc.tile_pool(name="strip", bufs=2) as strip,
            tc.tile_pool(name="accp", bufs=1) as accp,
        ):
            acc_all = accp.tile([P, NTR * NGROUPS], F32)
            eps_t = accp.tile([P, 1], F32)
            nc.vector.memset(eps_t[:], EPS)

            for rit in range(NTR):
                it = rit % NT
                pt = io.tile([P, CPP * D], F32, tag="pt")
                tt = io.tile([P, CPP * D], F32, tag="tt")
                nc.sync.dma_start(out=pt[:], in_=pred_v[it])
                nc.sync.dma_start(out=tt[:], in_=targ_v[it])

                p3 = pt[:].rearrange("p (c d) -> p c d", d=D)   # [128,CPP,30]
                t3 = tt[:].rearrange("p (c d) -> p c d", d=D)
                pb = pt[:].rearrange("p (c b f) -> p c b f", b=2 * 3, f=5)[:, :, 0:2, :]
                tb = tt[:].rearrange("p (c b f) -> p c b f", b=2 * 3, f=5)[:, :, 0:2, :]
                # pb/tb: [128, CPP, 2, 5] box view

                scratch = strip.tile([P, CPP, SW], BF16, tag="scratch")

                # ---- masks (exact {0,1} in bf16) ----
                obj = mid.tile([P, CPP, 1], BF16, tag="obj")
                noo = mid.tile([P, CPP, 1], BF16, tag="noo")
                t4 = t3[:, :, 4:5]
                nc.vector.tensor_scalar(out=obj[:], in0=t4, scalar1=0.0,
                                        scalar2=None, op0=Alu.is_gt)
                nc.vector.tensor_scalar(out=noo[:], in0=t4, scalar1=0.0,
                                        scalar2=None, op0=Alu.is_le)

                # ---- classes: (p-t)*obj into scratch[0:20] ----
                dcls = mid.tile([P, CPP, 20], BF16, tag="dcls")
                nc.vector.tensor_tensor(out=dcls[:], in0=p3[:, :, 10:30],
                                        in1=t3[:, :, 10:30], op=Alu.subtract)
                nc.vector.tensor_tensor(out=scratch[:, :, 0:20], in0=dcls[:],
                                        in1=bc(obj[:], 20), op=Alu.mult)

                # ---- noobj: (p-t)*noo on conf channels into scratch[22:24] ----
                d49 = mid.tile([P, CPP, 2], BF16, tag="d49")
                nc.vector.tensor_tensor(out=d49[:], in0=d1(pb[:, :, :, 4:5]),
                                        in1=d1(tb[:, :, :, 4:5]), op=Alu.subtract)
                nc.vector.tensor_tensor(out=scratch[:, :, 22:24], in0=d49[:],
                                        in1=bc(noo[:], 2), op=Alu.mult)

                # ---- xyxy for both tensors (packed [128, CPP, 2, 2] bf16) ----
                uvp = mid.tile([P, CPP, 2, 2], BF16, tag="uvp")
                uvt = mid.tile([P, CPP, 2, 2], BF16, tag="uvt")
                hwp = mid.tile([P, CPP, 2, 2], BF16, tag="hwp")
                hwt = mid.tile([P, CPP, 2, 2], BF16, tag="hwt")
                nc.scalar.mul(uvp[:], pb[:, :, :, 0:2], INV_S)
                nc.scalar.mul(uvt[:], tb[:, :, :, 0:2], INV_S)
                nc.scalar.mul(hwp[:], pb[:, :, :, 2:4], 0.5)
                nc.scalar.mul(hwt[:], tb[:, :, :, 2:4], 0.5)

                xy1p = mid.tile([P, CPP, 2, 2], BF16, tag="xy1p")
                xy2p = mid.tile([P, CPP, 2, 2], BF16, tag="xy2p")
                xy1t = mid.tile([P, CPP, 2, 2], BF16, tag="xy1t")
                xy2t = mid.tile([P, CPP, 2, 2], BF16, tag="xy2t")
                nc.vector.tensor_tensor(out=xy1p[:], in0=uvp[:], in1=hwp[:], op=Alu.subtract)
                nc.vector.tensor_tensor(out=xy2p[:], in0=uvp[:], in1=hwp[:], op=Alu.add)
                nc.vector.tensor_tensor(out=xy1t[:], in0=uvt[:], in1=hwt[:], op=Alu.subtract)
                nc.vector.tensor_tensor(out=xy2t[:], in0=uvt[:], in1=hwt[:], op=Alu.add)

                # areas (reference-exact: from xyxy differences)
                dxyp = mid.tile([P, CPP, 2, 2], BF16, tag="dxyp")
                dxyt = mid.tile([P, CPP, 2, 2], BF16, tag="dxyt")
                nc.vector.tensor_tensor(out=dxyp[:], in0=xy2p[:], in1=xy1p[:], op=Alu.subtract)
                nc.vector.tensor_tensor(out=dxyt[:], in0=xy2t[:], in1=xy1t[:], op=Alu.subtract)
                areap = mid.tile([P, CPP, 2], BF16, tag="areap")
                areat = mid.tile([P, CPP, 2], BF16, tag="areat")
                nc.vector.tensor_tensor(out=areap[:], in0=d1(dxyp[:, :, :, 0:1]),
                                        in1=d1(dxyp[:, :, :, 1:2]), op=Alu.mult)
                nc.vector.tensor_tensor(out=areat[:], in0=d1(dxyt[:, :, :, 0:1]),
                                        in1=d1(dxyt[:, :, :, 1:2]), op=Alu.mult)

                # ---- pairwise IoU; loop over target box j, pred boxes vectorized ----
                inter_a = mid.tile([P, CPP, 2, 2], F32, tag="inter")  # [.., j, i]
                union_a = mid.tile([P, CPP, 2, 2], F32, tag="union")
                for j in range(2):
                    xy1tj = ibc(xy1t[:, :, j, :], 2, 2)   # [P, CPP, 2(i), 2(ch)]
                    xy2tj = ibc(xy2t[:, :, j, :], 2, 2)
                    lt = mid.tile([P, CPP, 2, 2], BF16, tag="lt")
                    rb = mid.tile([P, CPP, 2, 2], BF16, tag="rb")
                    nc.vector.tensor_tensor(out=lt[:], in0=xy1p[:], in1=xy1tj, op=Alu.max)
                    nc.vector.tensor_tensor(out=rb[:], in0=xy2p[:], in1=xy2tj, op=Alu.min)
                    clip = mid.tile([P, CPP, 2, 2], BF16, tag="clip")
                    nc.vector.scalar_tensor_tensor(out=clip[:], in0=rb[:], scalar=0.0,
                                                   in1=lt[:], op0=Alu.bypass,
                                                   op1=Alu.subtract)
                    nc.vector.tensor_scalar(out=clip[:], in0=clip[:], scalar1=0.0,
                                            scalar2=None, op0=Alu.max)
                    nc.vector.tensor_tensor(out=inter_a[:, :, j, :],
                                            in0=d1(clip[:, :, :, 0:1]),
                                            in1=d1(clip[:, :, :, 1:2]), op=Alu.mult)
                    usum = mid.tile([P, CPP, 2], F32, tag="usum")
                    nc.vector.tensor_tensor(out=usum[:], in0=areap[:],
                                            in1=bc(areat[:, :, j:j + 1], 2), op=Alu.add)
                    nc.vector.tensor_tensor(out=union_a[:, :, j, :], in0=usum[:],
                                            in1=inter_a[:, :, j, :], op=Alu.subtract)
                # ---- argmax via cross-multiplication (no division):
                # iou1 > iou0  <=>  inter1*union0 > inter0*union1  (unions > 0)
                i0 = d1(inter_a[:, :, :, 0:1])
                i1 = d1(inter_a[:, :, :, 1:2])
                u0 = d1(union_a[:, :, :, 0:1])
                u1 = d1(union_a[:, :, :, 1:2])
                cr1 = mid.tile([P, CPP, 2], F32, tag="cr1")
                cr0 = mid.tile([P, CPP, 2], F32, tag="cr0")
                nc.vector.tensor_tensor(out=cr1[:], in0=i1, in1=u0, op=Alu.mult)
                nc.vector.tensor_tensor(out=cr0[:], in0=i0, in1=u1, op=Alu.mult)
                g = mid.tile([P, CPP, 2], F32, tag="g")    # 1.0 if pred box 1 wins
                nc.vector.tensor_tensor(out=g[:], in0=cr1[:], in1=cr0[:], op=Alu.is_gt)
                # select the winning (inter, union) and divide once (half-width
                # reciprocal: the DVE divide is ~20ns/elem on HW)
                di = mid.tile([P, CPP, 2], F32, tag="di")
                isel = mid.tile([P, CPP, 2], F32, tag="isel")
                du = mid.tile([P, CPP, 2], F32, tag="du")
                usel = mid.tile([P, CPP, 2], F32, tag="usel")
                nc.vector.tensor_tensor(out=di[:], in0=i1, in1=i0, op=Alu.subtract)
                nc.vector.tensor_tensor(out=di[:] if False else di[:], in0=di[:], in1=g[:], op=Alu.mult)
                nc.vector.tensor_tensor(out=isel[:], in0=i0, in1=di[:], op=Alu.add)
                nc.vector.tensor_tensor(out=du[:], in0=u1, in1=u0, op=Alu.subtract)
                nc.vector.tensor_tensor(out=du[:] if False else du[:], in0=du[:], in1=g[:], op=Alu.mult)
                nc.vector.tensor_tensor(out=usel[:], in0=u0, in1=du[:], op=Alu.add)
                rsel = mid.tile([P, CPP, 2], F32, tag="rsel")
                nc.vector.reciprocal(out=rsel[:].rearrange("p c j -> p (c j)"),
                                     in_=usel[:].rearrange("p c j -> p (c j)"))
                m = mid.tile([P, CPP, 2], F32, tag="m")    # max iou per target j
                nc.vector.tensor_tensor(out=m[:], in0=isel[:], in1=rsel[:], op=Alu.mult)

                # ---- conf targets (last-write-wins) ----
                m0, m1 = m[:, :, 0:1], m[:, :, 1:2]
                g0, g1 = g[:, :, 0:1], g[:, :, 1:2]
                dm = mid.tile([P, CPP, 1], F32, tag="dm")
                gdm = mid.tile([P, CPP, 1], F32, tag="gdm")
                ct = mid.tile([P, CPP, 2], BF16, tag="ct")
                nc.vector.tensor_tensor(out=dm[:], in0=m0, in1=m1, op=Alu.subtract)
                nc.vector.tensor_tensor(out=gdm[:], in0=g1, in1=dm[:], op=Alu.mult)
                nc.vector.tensor_tensor(out=ct[:, :, 0:1], in0=m1, in1=gdm[:], op=Alu.add)
                nc.vector.tensor_tensor(out=ct[:, :, 1:2], in0=m0, in1=gdm[:], op=Alu.subtract)

                # ---- responsibility masks (exact {0,1}) ----
                gg = mid.tile([P, CPP, 1], BF16, tag="gg")
                s01 = mid.tile([P, CPP, 1], BF16, tag="s01")
                rr = mid.tile([P, CPP, 2], BF16, tag="rr")
                nc.vector.tensor_tensor(out=gg[:], in0=g0, in1=g1, op=Alu.mult)
                nc.vector.tensor_tensor(out=s01[:], in0=g0, in1=g1, op=Alu.add)
                nc.vector.tensor_scalar(out=rr[:, :, 0:1], in0=gg[:], scalar1=-1.0,
                                        scalar2=1.0, op0=Alu.mult, op1=Alu.add)
                nc.vector.scalar_tensor_tensor(out=rr[:, :, 1:2], in0=gg[:], scalar=-1.0,
                                               in1=s01[:], op0=Alu.mult, op1=Alu.add)
                rm = mid.tile([P, CPP, 2], BF16, tag="rm")
                nc.vector.tensor_tensor(out=rm[:], in0=rr[:], in1=bc(obj[:], 2), op=Alu.mult)

                # ---- contain: (pconf - ct)*rm into scratch[20:22] ----
                pc2 = mid.tile([P, CPP, 2], BF16, tag="pc2")
                nc.vector.tensor_scalar_mul(pc2[:], d1(pb[:, :, :, 4:5]), 1.0)
                e = mid.tile([P, CPP, 2], BF16, tag="e")
                nc.vector.tensor_tensor(out=e[:], in0=pc2[:], in1=ct[:],
                                        op=Alu.subtract)
                nc.vector.tensor_tensor(out=scratch[:, :, 20:22], in0=e[:], in1=rm[:],
                                        op=Alu.mult)

                # ---- loc xy: (pxy - txy)*rm into scratch[24:28] ----
                dxy = mid.tile([P, CPP, 2, 2], BF16, tag="dxy")
                nc.vector.tensor_tensor(out=dxy[:], in0=pb[:, :, :, 0:2],
                                        in1=tb[:, :, :, 0:2], op=Alu.subtract)
                sxy = scratch[:, :, 24:28].rearrange("p c (b f) -> p c b f", b=2)
                nc.vector.tensor_tensor(out=sxy, in0=dxy[:], in1=abc(rm[:], 2), op=Alu.mult)

                # ---- loc wh: (sqrt(pwh+eps)-sqrt(twh+eps))*rm into scratch[28:32] ----
                sqp = mid.tile([P, CPP, 2, 2], BF16, tag="sqp")
                sqt = mid.tile([P, CPP, 2, 2], BF16, tag="sqt")
                nc.scalar.activation(out=sqp[:], in_=pb[:, :, :, 2:4], func=Act.Sqrt,
                                     bias=eps_t[:], scale=1.0)
                nc.scalar.activation(out=sqt[:], in_=tb[:, :, :, 2:4], func=Act.Sqrt,
                                     bias=eps_t[:], scale=1.0)
                dwh = mid.tile([P, CPP, 2, 2], BF16, tag="dwh")
                nc.vector.tensor_tensor(out=dwh[:], in0=sqp[:], in1=sqt[:], op=Alu.subtract)
                swh = scratch[:, :, 28:32].rearrange("p c (b f) -> p c b f", b=2)
                nc.vector.tensor_tensor(out=swh, in0=dwh[:], in1=abc(rm[:], 2), op=Alu.mult)

                # ---- fused square+sum per scale group (in place) ----
                seg1 = scratch[:, :, 0:22]
                seg2 = scratch[:, :, 22:24]
                seg3 = scratch[:, :, 24:32]
                base = rit * NGROUPS
                nc.scalar.activation(out=seg1, in_=seg1, func=Act.Square, scale=1.0,
                                     accum_out=acc_all[:, base:base + 1])
                nc.scalar.activation(out=seg2, in_=seg2, func=Act.Square, scale=SQRT_HALF,
                                     accum_out=acc_all[:, base + 1:base + 2])
                nc.scalar.activation(out=seg3, in_=seg3, func=Act.Square, scale=SQRT5,
                                     accum_out=acc_all[:, base + 2:base + 3])

            nc.sync.dma_start(out=out[:], in_=acc_all[:])

    split_sync_waits(nc)
    return nc


_NC_CACHE = None


def kernel(pred_tensor: np.ndarray, target_tensor: np.ndarray) -> np.ndarray:
    global _NC_CACHE
    if _NC_CACHE is None:
        _NC_CACHE = build_kernel()
    nc = _NC_CACHE

    p = np.ascontiguousarray(pred_tensor, dtype=np.float32).reshape(N_CORES, K_CORE, D)
    t = np.ascontiguousarray(target_tensor, dtype=np.float32).reshape(N_CORES, K_CORE, D)
    in_maps = [{"pred": p[i], "targ": t[i]} for i in range(N_CORES)]
    res = run_bass_kernel_spmd(nc, in_maps, core_ids=list(range(N_CORES)))
    total = 0.0
    for i in range(N_CORES):
        total += res.results[i]["out"].astype(np.float64).sum()
    return np.float32(total / BATCH)
